# revision 8
# baseline (speedup 1.0000x reference)
"""Causal self-attention block (QKV -> causal attention -> 2 projections)
distributed over 8 NeuronCores via Bass/Tile.

Sharding: tensor-parallel over heads (2 heads/core, both batches on every
core). The whole on-device computation runs in transposed activation space
(channel/head-dim on partitions, tokens on the free axis) so no on-device
transposes of activations are ever needed:

  phase 1: Q^T, K^T, V^T = w^T @ x^T  (x^T supplied pre-transposed by host)
           V^T is PE-transposed into V[k, d] tiles with an appended ones
           column (gives softmax row-sums for free in phase 2).
  phase 2: S^T = K Q^T per 128k x 512q chunk (block-causal, diagonal chunks
           masked after exp), P = exp(S^T) on ScalarE (no max subtraction:
           scores are ~N(0,1), fp16 has enough range), att^T[d+1, q] =
           V_ext^T P^T accumulated over k chunks on PE, normalized by the
           row-sum reciprocal broadcast via a rank-1 PE outer product.
           Interleaved into phase 1 (window w's attention jobs are emitted
           as soon as windows <= w are projected).
  A2A:     one 8-rank AllToAll (fp16) moves att^T head-blocks so each core
           owns 512 token rows of all 16 heads.
  phase 3: y^T = wp1^T att^T ; out^T = wp2^T y^T ; host un-transposes.

All tensors stream as fp16 (same PE rate as f32r on TRN2, 10-bit mantissa =
f32r-class precision, half the DMA/collective bytes, 2x DVE modes); PSUM
accumulation stays fp32. Elementwise work is spread over DVE + Pool(gpsimd)
to keep both off the critical path.
"""

import numpy as np

import concourse.bass as bass
import concourse.tile as tile
from concourse import bacc, mybir
from concourse.bass_utils import run_bass_kernel_spmd

B, T, C, H, D = 2, 2048, 1024, 16, 64
NCORES = 8
HPC = H // NCORES          # heads per core = 2
TT = B * T                 # 4096 flat (b, t) rows
W = 512                    # token window / q chunk
NW = TT // W               # 8 windows
WPB = T // W               # 4 windows per batch
QS = TT // NCORES          # 512 rows per core after A2A
NCI = C // 128             # 8 channel chunks
KC = 128                   # k chunk
F32 = mybir.dt.float32
F16 = mybir.dt.float16
F32R = mybir.dt.float32r
EXP = mybir.ActivationFunctionType.Exp

_NC_CACHE = {}

import os
KSTAGE = int(os.environ.get("KSTAGE", "4"))
KNW = int(os.environ.get("KNW", str(NW)))
KNOCC = int(os.environ.get("KNOCC", "0"))
KREP = int(os.environ.get("KREP", "1"))

# consts packing: fp16 tensor holds the causal masks; f32r tensor holds the
# PE-side constants (stationary operands must be 4-byte to stay self-loading)
CST_MASK = 0               # [128, 4*W] causal masks for rel 0..3
CST_F = 4 * W
CSR_VONE = 0               # [128, 8] ones (written into V_ext ones cols)
CSR_ID = 8                 # [128, 64] stacked identity (PE transpose)
CSR_ONE = 8 + 64           # [128, 64] all-ones (row 0 used as lhsT)
CSR_F = CSR_ONE + 64


def build_nc():
    if "nc" in _NC_CACHE:
        return _NC_CACHE["nc"]
    nc = bacc.Bacc("TRN2", target_bir_lowering=False, debug=False,
                   num_devices=NCORES)
    xt_in = nc.dram_tensor("xt", [128, NW, NCI, W], F16, kind="ExternalInput")
    wq_in = nc.dram_tensor("wq", [128, NCI, 128], F16, kind="ExternalInput")
    wk_in = nc.dram_tensor("wk", [128, NCI, 128], F16, kind="ExternalInput")
    wv_in = nc.dram_tensor("wv", [128, NCI, 128], F16, kind="ExternalInput")
    wp1_in = nc.dram_tensor("wp1", [128, NCORES, C], F16, kind="ExternalInput")
    wp2_in = nc.dram_tensor("wp2", [128, NCI, C], F16, kind="ExternalInput")
    cst_in = nc.dram_tensor("consts", [128, CST_F], F16, kind="ExternalInput")
    csr_in = nc.dram_tensor("constsr", [128, CSR_F], F16, kind="ExternalInput")
    out_t = nc.dram_tensor("out_t", [C, QS], F16, kind="ExternalOutput")

    with tile.TileContext(nc) as tc:
      for rep in range(KREP):
        with (
            tc.tile_pool(name=f"dramp{rep}", bufs=2, space="DRAM") as dramp,
            tc.tile_pool(name=f"misc{rep}", bufs=1) as misc,
            tc.tile_pool(name=f"wqkv{rep}", bufs=1) as wqkv,
            tc.tile_pool(name=f"xtp{rep}", bufs=3) as xtp,
            tc.tile_pool(name=f"vtp{rep}", bufs=2) as vtp,
            tc.tile_pool(name=f"qtp{rep}", bufs=NW) as qtp,
            tc.tile_pool(name=f"ktp{rep}", bufs=NW) as ktp,
            tc.tile_pool(name=f"vp{rep}", bufs=NW) as vp,
            tc.tile_pool(name=f"attp{rep}", bufs=1) as attp,
            tc.tile_pool(name=f"rcpool{rep}", bufs=4) as rcpool,
            tc.tile_pool(name=f"pbp{rep}", bufs=3) as pbp,
            tc.tile_pool(name=f"wpp{rep}", bufs=1) as wpp,
            tc.tile_pool(name=f"finp{rep}", bufs=3) as finp,
            tc.tile_pool(name=f"pp{rep}", bufs=2, space="PSUM") as pp,
            tc.tile_pool(name=f"pap{rep}", bufs=2, space="PSUM") as pap,
        ):
            bounce_in = dramp.tile([NCORES, 128, W], F16, name="bounce_in")
            bounce_out = dramp.tile([NCORES, 128, W], F16, name="bounce_out")

            wqt = wqkv.tile([128, NCI, 128], F16, name="wqt")
            wkt = wqkv.tile([128, NCI, 128], F16, name="wkt")
            wvt = wqkv.tile([128, NCI, 128], F16, name="wvt")
            csts = misc.tile([128, CST_F], F16, name="csts")
            masks_t = csts.rearrange("p (k f) -> p k f", f=W)
            csr = misc.tile([128, CSR_F], F16, name="csr")
            vones_t = csr[:, CSR_VONE:CSR_VONE + 8].rearrange(
                "p (k l) -> p k l", l=2)
            ident = csr[:, CSR_ID:CSR_ID + 64]
            ones_t = csr[0:1, CSR_ONE:CSR_ONE + 64]

            # attention output staged as one tile so the A2A feed is 1 DMA
            att_all = attp.tile([128, NW, W], F16, name="att_all")

            # projection weights (prefetched behind phase-1 weights)
            wp1t = wpp.tile([128, NCORES, C], F16, name="wp1t")
            wp2t = wpp.tile([128, NCI, C], F16, name="wp2t")

            # DMA order = priority order: phase-1 weights + consts first.
            for ci in range(4):
                nc.sync.dma_start(wqt[:, 2 * ci:2 * ci + 2, :],
                                  wq_in[:, 2 * ci:2 * ci + 2, :])
            nc.sync.dma_start(wkt[:], wk_in[:])
            nc.sync.dma_start(wvt[:], wv_in[:])
            nc.sync.dma_start(csts[:], cst_in[:])
            nc.sync.dma_start(csr[:], csr_in[:])

            qt_tiles, kt_tiles, v_tiles = [], [], []

            def qkv_window(w, xtw):
                """Project window w -> Q^T, K^T tiles (fp16) + V[k,d] tiles."""
                for fam, wt, dst in (("q", wqt, qt_tiles), ("k", wkt, kt_tiles)):
                    ps = pp.tile([128, W], F32, name=f"ps_{fam}{w}", tag="pp")
                    for ci in range(NCI):
                        nc.tensor.matmul(ps[:], wt[:, ci, :], xtw[:, ci, :],
                                         start=(ci == 0), stop=(ci == NCI - 1))
                    sb = (qtp if fam == "q" else ktp).tile(
                        [128, W], F16, name=f"{fam}t{w}", tag=f"{fam}t")
                    nc.vector.tensor_copy(sb[:], ps[:])
                    dst.append(sb)

                psv = pp.tile([128, W], F32, name=f"ps_v{w}", tag="pp")
                for ci in range(NCI):
                    nc.tensor.matmul(psv[:], wvt[:, ci, :], xtw[:, ci, :],
                                     start=(ci == 0), stop=(ci == NCI - 1))
                vtw = vtp.tile([128, W], F16, name="vtw", tag="vtw")
                nc.scalar.copy(vtw[:], psv[:])

                # V^T window -> V[k, d] tiles via PE transpose.
                # NB: PE writes at a free-offset inside a PSUM bank crash
                # the device; every transpose gets its own bank-aligned tile.
                vw = vp.tile([128, 4, 130], F16, name=f"vw{w}", tag="vw")
                for kcl in range(4):
                    for l in range(HPC):
                        pvt = pp.tile([128, 64], F16,
                                      name=f"pvt{w}_{kcl}{l}", tag="pp")
                        nc.tensor.transpose(
                            pvt[:],
                            vtw[64 * l:64 * l + 64, KC * kcl:KC * (kcl + 1)],
                            ident[64 * l:64 * l + 64, :])
                        nc.vector.tensor_copy(
                            vw.rearrange("p k (l e) -> p k l e",
                                         e=65)[:, kcl, l, 0:64],
                            pvt[:])
                nc.gpsimd.tensor_copy(
                    vw.rearrange("p k (l e) -> p k l e", e=65)[:, :, :, 64],
                    vones_t[:])
                v_tiles.append(vw)

            def emit_tail(tb, tj, tqw, tpsas):
                for l in range(HPC):
                    rc = rcpool.tile([1, W], F16, name=f"rc{tb}{l}{tj}",
                                     tag="rc")
                    with nc.allow_low_precision(reason="fp16 recip"):
                        nc.vector.reciprocal(rc[:], tpsas[l][64:65, :])
                    prep = pp.tile([64, W], F32, name=f"prep{tb}{l}{tj}",
                                   tag="pp")
                    nc.tensor.matmul(prep[:], ones_t[:], rc[:],
                                     start=True, stop=True)
                    dst = att_all[64 * l:64 * l + 64, tqw, :]
                    with nc.allow_low_precision(reason="fp16 att"):
                        nc.vector.tensor_copy(dst, tpsas[l][0:64, :])
                        nc.vector.tensor_mul(dst, dst, prep[:])

            pending = None

            def attention_window(b, j):
                nonlocal pending
                qw = WPB * b + j
                nk = 4 * (j + 1)
                psas = [pap.tile([65, W], F32, name=f"psa{b}{l}{j}",
                                 tag="psa") for l in range(HPC)]
                for kc2 in range(nk // 2):
                    kca, kcb = 2 * kc2, 2 * kc2 + 1
                    for l in range(HPC):
                        pss = pp.tile([128, 2, W], F32,
                                      name=f"pss{b}{l}{j}{kc2}",
                                      tag="pp2", bufs=2)
                        for half, kc in ((0, kca), (1, kcb)):
                            kw = WPB * b + kc // 4
                            kcl = kc % 4
                            nc.tensor.matmul(
                                pss[:, half, :],
                                kt_tiles[kw][64 * l:64 * l + 64,
                                             KC * kcl:KC * (kcl + 1)],
                                qt_tiles[qw][64 * l:64 * l + 64, :],
                                start=True, stop=True)
                        pb = pbp.tile([128, 2, W], F16,
                                      name=f"pb{b}{l}{j}{kc2}", tag="pb")
                        nc.scalar.activation(
                            pb.rearrange("p h f -> p (h f)"),
                            pss.rearrange("p h f -> p (h f)"), EXP)
                        for half, kc in ((0, kca), (1, kcb)):
                            rel = kc - 4 * j
                            if rel >= 0:
                                nc.vector.tensor_mul(
                                    pb[:, half, :], pb[:, half, :],
                                    masks_t[:, rel, :])
                        for half, kc in ((0, kca), (1, kcb)):
                            kw = WPB * b + kc // 4
                            kcl = kc % 4
                            nc.tensor.matmul(
                                psas[l][:],
                                v_tiles[kw][:, kcl, 65 * l:65 * l + 65],
                                pb[:, half, :],
                                start=(kc == 0), stop=(kc == nk - 1))
                    if kc2 == 0 and pending is not None:
                        emit_tail(*pending)
                        pending = None
                if pending is not None:
                    emit_tail(*pending)
                pending = (b, j, qw, psas)

            # ---------- phases 1+2 interleaved ----------
            # window w's attention is emitted after window w+1's projection:
            # its Q/K/V copies complete while the next window's QKV matmuls
            # run, so the score matmuls never stall the PE queue head.
            for w in range(KNW):
                xtw = xtp.tile([128, NCI, W], F16, name="xtw", tag="xtw")
                if w == 0:
                    # split the first window per channel-chunk pair so the
                    # first projection matmul starts after 256KB, not 1MB
                    for ci in range(4):
                        nc.sync.dma_start(xtw[:, 2 * ci:2 * ci + 2, :],
                                          xt_in[:, w, 2 * ci:2 * ci + 2, :])
                else:
                    nc.sync.dma_start(xtw[:], xt_in[:, w, :, :])
                if w == 1:
                    nc.sync.dma_start(wp1t[:], wp1_in[:])
                    nc.sync.dma_start(wp2t[:], wp2_in[:])
                qkv_window(w, xtw)
                if KSTAGE >= 2 and w >= 1:
                    attention_window((w - 1) // WPB, (w - 1) % WPB)
            if KSTAGE >= 2 and KNW == NW:
                attention_window(B - 1, WPB - 1)

            if KSTAGE == 1:
                for w in range(KNW):
                    nc.sync.dma_start(
                        out_t.ap().rearrange("(e p) f -> e p f", p=128)[w],
                        qt_tiles[w][:])
            if pending is not None:
                emit_tail(*pending)
                pending = None

            # ---------- A2A ----------
            if KSTAGE == 2:
                nc.sync.dma_start(
                    out_t.ap().rearrange("(e p) f -> p e f", p=128),
                    att_all[:])
            if KSTAGE >= 3:
                nc.sync.dma_start(
                    bounce_in.rearrange("r p f -> p r f"), att_all[:])
                if not KNOCC:
                    nc.gpsimd.collective_compute(
                        "AllToAll", mybir.AluOpType.bypass,
                        ins=[bounce_in.opt()], outs=[bounce_out.opt()],
                        replica_groups=[list(range(NCORES))])

            # ---------- phase 3: output projections ----------
            if KSTAGE >= 3:
                rt = finp.tile([128, NCORES, QS], F16, name="rt", tag="fin")
                rt_src = bounce_in if KNOCC else bounce_out
                nc.sync.dma_start(rt[:], rt_src.rearrange("r p f -> p r f"))
                if KSTAGE == 3:
                    nc.sync.dma_start(
                        out_t.ap().rearrange("(e p) f -> p e f", p=128),
                        rt[:])
                else:
                    yt = finp.tile([128, NCI, QS], F16, name="yt", tag="fin")
                    for cc in range(NCI):
                        psy = pp.tile([128, QS], F32, name=f"psy{cc}",
                                      tag="pp")
                        for s in range(NCORES):
                            nc.tensor.matmul(
                                psy[:], wp1t[:, s, KC * cc:KC * (cc + 1)],
                                rt[:, s, :],
                                start=(s == 0), stop=(s == NCORES - 1))
                        nc.vector.tensor_copy(yt[:, cc, :], psy[:])

                    out_r = out_t.ap().rearrange("(e p) f -> e p f", p=128)
                    for ec in range(NCI):
                        pso = pp.tile([128, QS], F32, name=f"pso{ec}",
                                      tag="pp")
                        for cc in range(NCI):
                            nc.tensor.matmul(
                                pso[:], wp2t[:, cc, KC * ec:KC * (ec + 1)],
                                yt[:, cc, :],
                                start=(cc == 0), stop=(cc == NCI - 1))
                        ot = finp.tile([128, QS], F16, name=f"ot{ec}",
                                       tag="ot", bufs=2)
                        with nc.allow_low_precision(reason="fp16 out"):
                            if ec % 2 == 0:
                                nc.vector.tensor_copy(ot[:], pso[:])
                            else:
                                nc.scalar.copy(ot[:], pso[:])
                        nc.sync.dma_start(out_r[ec], ot[:])

    nc.compile()
    _NC_CACHE["nc"] = nc
    return nc


def prep_inputs(x, wq, wk, wv, wp1, wp2):
    """Host-side sharding / layout prep. Returns per-core input dicts."""
    x = np.asarray(x, np.float32)
    wq = np.asarray(wq, np.float32)
    wk = np.asarray(wk, np.float32)
    wv = np.asarray(wv, np.float32)
    wp1 = np.asarray(wp1, np.float32)
    wp2 = np.asarray(wp2, np.float32)

    # x^T in [p, w, ci, f] layout (fp16)
    xtf = x.reshape(TT, C).T                      # [C, TT]
    xt_host = np.ascontiguousarray(
        xtf.reshape(NCI, 128, NW, W).transpose(1, 2, 0, 3)).astype(np.float16)

    scale = 1.0 / np.sqrt(D)

    def wlay(wm):                                  # [C, 128] -> [128, NCI, 128]
        return np.ascontiguousarray(
            wm.reshape(NCI, 128, 128).transpose(1, 0, 2)).astype(np.float16)

    wp1_host = np.ascontiguousarray(
        wp1.reshape(C, C).reshape(NCORES, 128, C).transpose(1, 0, 2)).astype(
            np.float16)
    wp2_host = np.ascontiguousarray(
        wp2.reshape(NCI, 128, C).transpose(1, 0, 2)).astype(np.float16)

    p = np.arange(128)[:, None]
    f = np.arange(W)[None, :]
    cst_host = np.zeros((128, CST_F), np.float16)
    for rel in range(4):
        cst_host[:, CST_MASK + W * rel:CST_MASK + W * (rel + 1)] = (
            p <= f - 128 * rel).astype(np.float16)
    csr_host = np.zeros((128, CSR_F), np.float16)
    csr_host[:, CSR_VONE:CSR_VONE + 8] = 1.0
    csr_host[:, CSR_ID:CSR_ID + 64] = np.concatenate(
        [np.eye(64, dtype=np.float16)] * 2, axis=0)
    csr_host[:, CSR_ONE:CSR_ONE + 64] = 1.0

    in_maps = []
    for c in range(NCORES):
        h0 = HPC * c
        wq_c = wlay(wq[:, h0:h0 + HPC, :].reshape(C, HPC * D) * scale)
        wk_c = wlay(wk[:, h0:h0 + HPC, :].reshape(C, HPC * D))
        wv_c = wlay(wv[:, h0:h0 + HPC, :].reshape(C, HPC * D))
        in_maps.append({
            "xt": xt_host, "wq": wq_c, "wk": wk_c, "wv": wv_c,
            "wp1": wp1_host, "wp2": wp2_host, "consts": cst_host,
            "constsr": csr_host,
        })
    return in_maps


def assemble_output(results):
    out = np.empty((TT, C), np.float32)
    for r in range(NCORES):
        out[QS * r:QS * (r + 1), :] = results[r]["out_t"].astype(np.float32).T
    return out.reshape(B, T, C)


def kernel(x, wq, wk, wv, wp1, wp2):
    in_maps = prep_inputs(x, wq, wk, wv, wp1, wp2)
    nc = build_nc()
    res = run_bass_kernel_spmd(nc, in_maps, list(range(NCORES)))
    return assemble_output(res.results)


# revision 15
# speedup vs baseline: 1.1218x; 1.1218x over previous
"""Causal self-attention block (QKV -> causal attention -> 2 projections)
distributed over 8 NeuronCores via Bass/Tile.

Sharding: tensor-parallel over heads (2 heads/core, both batches on every
core). The whole on-device computation runs in transposed activation space
(channel/head-dim on partitions, tokens on the free axis) so no on-device
transposes of activations are ever needed:

  phase 1: Q^T, K^T, V^T = w^T @ x^T  (x^T supplied pre-transposed by host)
           V^T is PE-transposed into V[k, d] tiles with an appended ones
           column (gives softmax row-sums for free in phase 2).
  phase 2: S^T = K Q^T per 128k x 512q chunk (block-causal, diagonal chunks
           masked after exp), P = exp(S^T) on ScalarE (no max subtraction:
           scores are ~N(0,1), fp16 has enough range), att^T[d+1, q] =
           V_ext^T P^T accumulated over k chunks on PE, normalized by the
           row-sum reciprocal broadcast via a rank-1 PE outer product.
           Interleaved into phase 1 (window w's attention jobs are emitted
           as soon as windows <= w are projected).
  A2A:     one 8-rank AllToAll (fp16) moves att^T head-blocks so each core
           owns 512 token rows of all 16 heads.
  phase 3: y^T = wp1^T att^T ; out^T = wp2^T y^T ; host un-transposes.

All tensors stream as fp16 (same PE rate as f32r on TRN2, 10-bit mantissa =
f32r-class precision, half the DMA/collective bytes, 2x DVE modes); PSUM
accumulation stays fp32. Elementwise work is spread over DVE + Pool(gpsimd)
to keep both off the critical path.
"""

import numpy as np

import concourse.bass as bass
import concourse.tile as tile
from concourse import bacc, mybir
from concourse.bass_utils import run_bass_kernel_spmd

B, T, C, H, D = 2, 2048, 1024, 16, 64
NCORES = 8
HPC = H // NCORES          # heads per core = 2
TT = B * T                 # 4096 flat (b, t) rows
W = 512                    # token window / q chunk
NW = TT // W               # 8 windows
WPB = T // W               # 4 windows per batch
QS = TT // NCORES          # 512 rows per core after A2A
NCI = C // 128             # 8 channel chunks
KC = 128                   # k chunk
F32 = mybir.dt.float32
F16 = mybir.dt.float16
F32R = mybir.dt.float32r
EXP = mybir.ActivationFunctionType.Exp

_NC_CACHE = {}

import os
KSTAGE = int(os.environ.get("KSTAGE", "4"))
KNW = int(os.environ.get("KNW", str(NW)))
KNOCC = int(os.environ.get("KNOCC", "0"))
KREP = int(os.environ.get("KREP", "1"))

# consts packing: fp16 tensor (masks + V-side constants), f32r tensor (ones
# row for the rank-1 reciprocal broadcast, matching rc's f32r dtype)
CST_MASK = 0               # [128, 4*W] causal masks for rel 0..3
CST_VONE = 4 * W           # [128, 8] ones (written into V_ext ones cols)
CST_ID = 4 * W + 8         # [128, 64] stacked identity (PE transpose)
CST_F = CST_ID + 64
CSR_ONE = 0                # [128, 64] all-ones (row 0 used as lhsT)
CSR_F = 64


def build_nc():
    if "nc" in _NC_CACHE:
        return _NC_CACHE["nc"]
    nc = bacc.Bacc("TRN2", target_bir_lowering=False, debug=False,
                   num_devices=NCORES)
    xt_in = nc.dram_tensor("xt", [128, NW, NCI, W], F32R, kind="ExternalInput")
    wq_in = nc.dram_tensor("wq", [128, NCI, 128], F32R, kind="ExternalInput")
    wk_in = nc.dram_tensor("wk", [128, NCI, 128], F32R, kind="ExternalInput")
    wv_in = nc.dram_tensor("wv", [128, NCI, 128], F32R, kind="ExternalInput")
    wp1_in = nc.dram_tensor("wp1", [128, NCORES, C], F16, kind="ExternalInput")
    wp2_in = nc.dram_tensor("wp2", [128, NCI, C], F32R, kind="ExternalInput")
    cst_in = nc.dram_tensor("consts", [128, CST_F], F16, kind="ExternalInput")
    csr_in = nc.dram_tensor("constsr", [128, CSR_F], F32R, kind="ExternalInput")
    out_t = nc.dram_tensor("out_t", [C, QS], F32, kind="ExternalOutput")

    with tile.TileContext(nc) as tc:
      for rep in range(KREP):
        with (
            tc.tile_pool(name=f"dramp{rep}", bufs=2, space="DRAM") as dramp,
            tc.tile_pool(name=f"misc{rep}", bufs=1) as misc,
            tc.tile_pool(name=f"wqkv{rep}", bufs=1) as wqkv,
            tc.tile_pool(name=f"xtp{rep}", bufs=3) as xtp,
            tc.tile_pool(name=f"vtp{rep}", bufs=2) as vtp,
            tc.tile_pool(name=f"qtp{rep}", bufs=NW) as qtp,
            tc.tile_pool(name=f"ktp{rep}", bufs=NW) as ktp,
            tc.tile_pool(name=f"vp{rep}", bufs=NW) as vp,
            tc.tile_pool(name=f"attp{rep}", bufs=1) as attp,
            tc.tile_pool(name=f"rcpool{rep}", bufs=4) as rcpool,
            tc.tile_pool(name=f"pbp{rep}", bufs=3) as pbp,
            tc.tile_pool(name=f"wpp{rep}", bufs=1) as wpp,
            tc.tile_pool(name=f"finp{rep}", bufs=3) as finp,
            tc.tile_pool(name=f"pp{rep}", bufs=2, space="PSUM") as pp,
            tc.tile_pool(name=f"pap{rep}", bufs=2, space="PSUM") as pap,
        ):
            bounce_in = dramp.tile([NCORES, 128, W], F16, name="bounce_in")
            bounce_out = dramp.tile([NCORES, 128, W], F16, name="bounce_out")

            wqt = wqkv.tile([128, NCI, 128], F32R, name="wqt")
            wkt = wqkv.tile([128, NCI, 128], F32R, name="wkt")
            wvt = wqkv.tile([128, NCI, 128], F32R, name="wvt")
            csts = misc.tile([128, CST_F], F16, name="csts")
            masks_t = csts[:, CST_MASK:CST_MASK + 4 * W].rearrange(
                "p (k f) -> p k f", f=W)
            vones_t = csts[:, CST_VONE:CST_VONE + 8].rearrange(
                "p (k l) -> p k l", l=2)
            ident = csts[:, CST_ID:CST_ID + 64]
            csr = misc.tile([128, CSR_F], F32R, name="csr")
            ones_t = csr[0:1, CSR_ONE:CSR_ONE + 64]

            # attention output staged as one tile so the A2A feed is 1 DMA
            att_all = attp.tile([128, NW, W], F16, name="att_all")

            # projection weights (prefetched behind phase-1 weights)
            wp1t = wpp.tile([128, NCORES, C], F16, name="wp1t")
            wp2t = wpp.tile([128, NCI, C], F32R, name="wp2t")

            # DMA order = priority order: interleave the first window's x
            # chunks with the weights so the first QKV matmul starts ~3us in.
            x0 = xtp.tile([128, NCI, W], F32R, name="xtw", tag="xtw")
            nc.sync.dma_start(wqt[:], wq_in[:])
            nc.sync.dma_start(x0[:, 0:2, :], xt_in[:, 0, 0:2, :])
            nc.sync.dma_start(wkt[:], wk_in[:])
            nc.sync.dma_start(x0[:, 2:4, :], xt_in[:, 0, 2:4, :])
            nc.sync.dma_start(wvt[:], wv_in[:])
            nc.sync.dma_start(x0[:, 4:6, :], xt_in[:, 0, 4:6, :])
            nc.sync.dma_start(csts[:], cst_in[:])
            nc.sync.dma_start(x0[:, 6:8, :], xt_in[:, 0, 6:8, :])
            nc.sync.dma_start(csr[:], csr_in[:])

            qt_tiles, kt_tiles, v_tiles = [], [], []

            def qkv_window(w, xtw):
                """Project window w -> Q^T, K^T tiles (fp16) + V[k,d] tiles."""
                for fam, wt, dst in (("q", wqt, qt_tiles), ("k", wkt, kt_tiles)):
                    ps = pp.tile([128, W], F32, name=f"ps_{fam}{w}", tag="pp")
                    for ci in range(NCI):
                        nc.tensor.matmul(ps[:], wt[:, ci, :], xtw[:, ci, :],
                                         start=(ci == 0), stop=(ci == NCI - 1))
                    sb = (qtp if fam == "q" else ktp).tile(
                        [128, W], F32R, name=f"{fam}t{w}", tag=f"{fam}t")
                    nc.vector.tensor_copy(sb[:], ps[:])
                    dst.append(sb)

                psv = pp.tile([128, W], F32, name=f"ps_v{w}", tag="pp")
                for ci in range(NCI):
                    nc.tensor.matmul(psv[:], wvt[:, ci, :], xtw[:, ci, :],
                                     start=(ci == 0), stop=(ci == NCI - 1))
                vtw = vtp.tile([128, W], F16, name="vtw", tag="vtw")
                nc.scalar.copy(vtw[:], psv[:])

                # V^T window -> V[k, d] tiles via PE transpose.
                # NB: PE writes at a free-offset inside a PSUM bank crash
                # the device; every transpose gets its own bank-aligned tile.
                vw = vp.tile([128, 4, 130], F16, name=f"vw{w}", tag="vw")
                for kcl in range(4):
                    for l in range(HPC):
                        pvt = pp.tile([128, 64], F16,
                                      name=f"pvt{w}_{kcl}{l}", tag="pp")
                        nc.tensor.transpose(
                            pvt[:],
                            vtw[64 * l:64 * l + 64, KC * kcl:KC * (kcl + 1)],
                            ident[64 * l:64 * l + 64, :])
                        nc.vector.tensor_copy(
                            vw.rearrange("p k (l e) -> p k l e",
                                         e=65)[:, kcl, l, 0:64],
                            pvt[:])
                nc.gpsimd.tensor_copy(
                    vw.rearrange("p k (l e) -> p k l e", e=65)[:, :, :, 64],
                    vones_t[:])
                v_tiles.append(vw)

            def emit_tail(tb, tj, tqw, tpsas):
                for l in range(HPC):
                    rc = rcpool.tile([1, W], F32R, name=f"rc{tb}{l}{tj}",
                                     tag="rc")
                    with nc.allow_low_precision(reason="fp16 recip"):
                        nc.vector.reciprocal(rc[:], tpsas[l][64:65, :])
                    prep = pp.tile([64, W], F32, name=f"prep{tb}{l}{tj}",
                                   tag="pp")
                    nc.tensor.matmul(prep[:], ones_t[:], rc[:],
                                     start=True, stop=True)
                    dst = att_all[64 * l:64 * l + 64, tqw, :]
                    with nc.allow_low_precision(reason="fp16 att"):
                        nc.vector.tensor_copy(dst, tpsas[l][0:64, :])
                        nc.vector.tensor_mul(dst, dst, prep[:])

            pending = None

            def attention_window(b, j):
                nonlocal pending
                qw = WPB * b + j
                nk = 4 * (j + 1)
                psas = [pap.tile([65, W], F32, name=f"psa{b}{l}{j}",
                                 tag="psa") for l in range(HPC)]
                for kc2 in range(nk // 2):
                    kca, kcb = 2 * kc2, 2 * kc2 + 1
                    for l in range(HPC):
                        pss = pp.tile([128, 2, W], F32,
                                      name=f"pss{b}{l}{j}{kc2}",
                                      tag="pp2", bufs=2)
                        for half, kc in ((0, kca), (1, kcb)):
                            kw = WPB * b + kc // 4
                            kcl = kc % 4
                            nc.tensor.matmul(
                                pss[:, half, :],
                                kt_tiles[kw][64 * l:64 * l + 64,
                                             KC * kcl:KC * (kcl + 1)],
                                qt_tiles[qw][64 * l:64 * l + 64, :],
                                start=True, stop=True)
                        pb = pbp.tile([128, 2, W], F16,
                                      name=f"pb{b}{l}{j}{kc2}", tag="pb")
                        nc.scalar.activation(
                            pb.rearrange("p h f -> p (h f)"),
                            pss.rearrange("p h f -> p (h f)"), EXP)
                        for half, kc in ((0, kca), (1, kcb)):
                            rel = kc - 4 * j
                            if rel >= 0:
                                nc.vector.tensor_mul(
                                    pb[:, half, :], pb[:, half, :],
                                    masks_t[:, rel, :])
                        for half, kc in ((0, kca), (1, kcb)):
                            kw = WPB * b + kc // 4
                            kcl = kc % 4
                            nc.tensor.matmul(
                                psas[l][:],
                                v_tiles[kw][:, kcl, 65 * l:65 * l + 65],
                                pb[:, half, :],
                                start=(kc == 0), stop=(kc == nk - 1))
                    if kc2 == 0 and pending is not None:
                        emit_tail(*pending)
                        pending = None
                if pending is not None:
                    emit_tail(*pending)
                pending = (b, j, qw, psas)

            # ---------- phases 1+2 interleaved ----------
            # window w's attention is emitted after window w+1's projection:
            # its Q/K/V copies complete while the next window's QKV matmuls
            # run, so the score matmuls never stall the PE queue head.
            for w in range(KNW):
                if w == 0:
                    xtw = x0
                else:
                    xtw = xtp.tile([128, NCI, W], F32R, name="xtw", tag="xtw")
                    nc.sync.dma_start(xtw[:], xt_in[:, w, :, :])
                if w == 6:
                    # projection weights: late enough not to contend with the
                    # x stream, early enough to beat phase 3
                    nc.sync.dma_start(wp1t[:], wp1_in[:])
                    nc.sync.dma_start(wp2t[:], wp2_in[:])
                qkv_window(w, xtw)
                if KSTAGE >= 2 and w >= 1:
                    attention_window((w - 1) // WPB, (w - 1) % WPB)
            if KSTAGE >= 2 and KNW == NW:
                attention_window(B - 1, WPB - 1)

            if KSTAGE == 1:
                for w in range(KNW):
                    nc.sync.dma_start(
                        out_t.ap().rearrange("(e p) f -> e p f", p=128)[w],
                        qt_tiles[w][:])
            if pending is not None:
                emit_tail(*pending)
                pending = None

            # ---------- A2A ----------
            if KSTAGE == 2:
                nc.sync.dma_start(
                    out_t.ap().rearrange("(e p) f -> p e f", p=128),
                    att_all[:])
            if KSTAGE >= 3:
                # per-window bounce feeds: each window's attention block
                # streams to DRAM as soon as it is normalized, so only the
                # last window's 128KB is left after phase 2 ends.
                for r in range(NCORES):
                    nc.sync.dma_start(bounce_in[r, :, :], att_all[:, r, :])
                if not KNOCC:
                    nc.gpsimd.collective_compute(
                        "AllToAll", mybir.AluOpType.bypass,
                        ins=[bounce_in.opt()], outs=[bounce_out.opt()],
                        replica_groups=[list(range(NCORES))])

            # ---------- phase 3: output projections ----------
            if KSTAGE >= 3:
                rt = finp.tile([128, NCORES, QS], F16, name="rt", tag="rt", bufs=1)
                rt_src = bounce_in if KNOCC else bounce_out
                for s in range(NCORES):
                    nc.sync.dma_start(rt[:, s, :], rt_src[s, :, :])
                if KSTAGE == 3:
                    nc.sync.dma_start(
                        out_t.ap().rearrange("(e p) f -> p e f", p=128),
                        rt[:])
                else:
                    yt = finp.tile([128, NCI, QS], F32R, name="yt", tag="yt", bufs=1)
                    for cc in range(NCI):
                        psy = pp.tile([128, QS], F32, name=f"psy{cc}",
                                      tag="pp")
                        for s in range(NCORES):
                            nc.tensor.matmul(
                                psy[:], wp1t[:, s, KC * cc:KC * (cc + 1)],
                                rt[:, s, :],
                                start=(s == 0), stop=(s == NCORES - 1))
                        nc.vector.tensor_copy(yt[:, cc, :], psy[:])

                    out_r = out_t.ap().rearrange("(e p) f -> e p f", p=128)
                    for ec in range(NCI):
                        pso = pp.tile([128, QS], F32, name=f"pso{ec}",
                                      tag="pp")
                        for cc in range(NCI):
                            nc.tensor.matmul(
                                pso[:], wp2t[:, cc, KC * ec:KC * (ec + 1)],
                                yt[:, cc, :],
                                start=(cc == 0), stop=(cc == NCI - 1))
                        ot = finp.tile([128, QS], F32, name=f"ot{ec}",
                                       tag="ot", bufs=2)
                        with nc.allow_low_precision(reason="fp16 out"):
                            if ec % 2 == 0:
                                nc.vector.tensor_copy(ot[:], pso[:])
                            else:
                                nc.scalar.copy(ot[:], pso[:])
                        nc.sync.dma_start(out_r[ec], ot[:])

    nc.compile()
    _NC_CACHE["nc"] = nc
    return nc


def prep_inputs(x, wq, wk, wv, wp1, wp2):
    """Host-side sharding / layout prep. Returns per-core input dicts."""
    x = np.asarray(x, np.float32)
    wq = np.asarray(wq, np.float32)
    wk = np.asarray(wk, np.float32)
    wv = np.asarray(wv, np.float32)
    wp1 = np.asarray(wp1, np.float32)
    wp2 = np.asarray(wp2, np.float32)

    # x^T in [p, w, ci, f] layout (fp16)
    xtf = x.reshape(TT, C).T                      # [C, TT]
    xt_host = np.ascontiguousarray(
        xtf.reshape(NCI, 128, NW, W).transpose(1, 2, 0, 3))

    scale = 1.0 / np.sqrt(D)

    def wlay(wm):                                  # [C, 128] -> [128, NCI, 128]
        return np.ascontiguousarray(
            wm.reshape(NCI, 128, 128).transpose(1, 0, 2))

    wp1_host = np.ascontiguousarray(
        wp1.reshape(C, C).reshape(NCORES, 128, C).transpose(1, 0, 2)).astype(
            np.float16)
    wp2_host = np.ascontiguousarray(
        wp2.reshape(NCI, 128, C).transpose(1, 0, 2))

    p = np.arange(128)[:, None]
    f = np.arange(W)[None, :]
    cst_host = np.zeros((128, CST_F), np.float16)
    for rel in range(4):
        cst_host[:, CST_MASK + W * rel:CST_MASK + W * (rel + 1)] = (
            p <= f - 128 * rel).astype(np.float16)
    cst_host[:, CST_VONE:CST_VONE + 8] = 1.0
    cst_host[:, CST_ID:CST_ID + 64] = np.concatenate(
        [np.eye(64, dtype=np.float16)] * 2, axis=0)
    csr_host = np.zeros((128, CSR_F), np.float32)
    csr_host[:, CSR_ONE:CSR_ONE + 64] = 1.0

    in_maps = []
    for c in range(NCORES):
        h0 = HPC * c
        wq_c = wlay(wq[:, h0:h0 + HPC, :].reshape(C, HPC * D) * scale)
        wk_c = wlay(wk[:, h0:h0 + HPC, :].reshape(C, HPC * D))
        wv_c = wlay(wv[:, h0:h0 + HPC, :].reshape(C, HPC * D))
        in_maps.append({
            "xt": xt_host, "wq": wq_c, "wk": wk_c, "wv": wv_c,
            "wp1": wp1_host, "wp2": wp2_host, "consts": cst_host,
            "constsr": csr_host,
        })
    return in_maps


def assemble_output(results):
    out = np.empty((TT, C), np.float32)
    for r in range(NCORES):
        out[QS * r:QS * (r + 1), :] = results[r]["out_t"].T
    return out.reshape(B, T, C)


def kernel(x, wq, wk, wv, wp1, wp2):
    in_maps = prep_inputs(x, wq, wk, wv, wp1, wp2)
    nc = build_nc()
    res = run_bass_kernel_spmd(nc, in_maps, list(range(NCORES)))
    return assemble_output(res.results)
